# revision 1
# baseline (speedup 1.0000x reference)
"""InterpretableMultiHeadAttention on 8 Trainium2 NeuronCores.

Model (reference): qkv = x @ W_qkv; 16 q/k heads of 64, one shared v head;
causal softmax attention per head with shared V; mean over heads; @ W_out.

Sharding: core = (batch b, head-group hg of 8 heads). Each core computes its
8 heads' attention for its batch, applies (W_out/16) to the head-sum, and the
host adds the two head-group partials per batch.

Layout strategy (all on-chip matmuls consume/produce transposed tensors so no
on-device transposes are needed; host pre-transposes x):
  qT/kT   [dh, t]  <- lhsT = W-slice [d, cols], rhs = xT [d, t]
  scoresT [s, t]   <- lhsT = kT [dh, s-tile],  rhs = qT [dh, t]   (K = 64)
  expT = exp(scoresT/8); causal mask applied in [s, t] layout
  attnT+denom [65, t] <- lhsT = v_aug [s, 65] (ones col), rhs = expT [s, t]
  out [t, m]       <- lhsT = accT [dh, t-tile], rhs = W_out/16 [dh, m]
No softmax max-subtraction: scores/8 ~ N(0,1) so exp is well-bounded.
"""

import numpy as np

import concourse.bass as bass
import concourse.mybir as mybir
import concourse.tile as tile
from concourse.bass_utils import run_bass_kernel_spmd
from concourse.masks import make_upper_triangular

F32 = mybir.dt.float32
# float32r: TF32-style reduced-precision fp32 matmul, 4x faster at N>=256.
MM = mybir.dt.float32r  # TF32-style: 4x faster matmul, ~1e-4 rel err

B, T, D = 4, 2048, 1024
H, DH = 16, 64
HPC = 8          # heads per core
NPAIR = HPC // 2
DCH = D // 128   # 8 contraction chunks
TCH = T // 512   # 4 free-dim chunks
TT = T // 128    # 16 token tiles
N_CORES = 8

_uid = [0]


def _split_multiwaits(nc, maxw=1):
    """walrus rejects instructions with multiple sync waits (observed on the
    Tile exit drain). Move extra waits onto same-engine NoOps just before."""
    for _name, bbh in nc.bb_map.items():
        bb = bbh.bb if hasattr(bbh, "bb") else bbh
        insts = bb.instructions
        new = []
        for inst in insts:
            si = inst.sync_info
            if si is not None and len(si.on_wait) > maxw:
                waits = list(si.on_wait)
                extra, keep = waits[:-maxw], waits[-maxw:]
                for k in range(0, len(extra), maxw):
                    _uid[0] += 1
                    nop = mybir.InstNoOp(
                        name=f"I-waitsplit-{_uid[0]}", ins=[], outs=[]
                    )
                    nop.engine = inst.engine
                    nop.sync_info = mybir.SyncInfo(
                        on_wait=extra[k : k + maxw], on_update=[]
                    )
                    new.append(nop)
                inst.sync_info = mybir.SyncInfo(
                    on_wait=keep, on_update=list(si.on_update)
                )
            new.append(inst)
        insts[:] = new


def _emit_body(nc, tc, xT, wqk, wv, wout, out):
    Exp = mybir.ActivationFunctionType.Exp
    ts = bass.ts

    from contextlib import ExitStack

    _ctx = ExitStack()
    consts = _ctx.enter_context(tc.tile_pool(name="consts", bufs=1))
    mask = consts.tile([128, 128], F32)           # 1.0 where col >= row
    make_upper_triangular(nc, mask, val=1.0, diag=True)
    wout_sb = consts.tile([64, D], MM)
    nc.sync.dma_start(out=wout_sb, in_=wout[:])
    ones64 = consts.tile([1, 64], MM)
    nc.vector.memset(ones64.bitcast(F32), 1.0)
    v_sb = consts.tile([128, TT, 65], MM)        # v tiles + ones column
    acc = consts.tile([64, T], MM)               # sum_h attn_h/d_h (T-major)
    nc.vector.memset(acc.bitcast(F32), 0.0)
    qk_sb = consts.tile([128, 8, T], MM)         # 4 q-pair + 4 k-pair tiles

    # ---- stage B: qkT + v projections (xT and W resident only here) ----
    with (
        tc.tile_pool(name="xt", bufs=1) as xt_pool,
        tc.tile_pool(name="w", bufs=1) as w_pool,
        tc.tile_pool(name="psqk", bufs=3, space="PSUM") as psqk,
        tc.tile_pool(name="psv", bufs=2, space="PSUM") as psv,
    ):
        xt_sb = xt_pool.tile([128, DCH, T], MM)
        wqk_sb = w_pool.tile([128, DCH, 1024], MM)
        wv_sb = w_pool.tile([128, DCH, 64], MM)
        for dc in range(DCH):
            nc.sync.dma_start(out=xt_sb[:, dc, :], in_=xT[ts(dc, 128), :])
            nc.sync.dma_start(out=wqk_sb[:, dc, :], in_=wqk[ts(dc, 128), :])
            nc.sync.dma_start(out=wv_sb[:, dc, :], in_=wv[ts(dc, 128), :])

        for ct in range(8):
            for tc_ in range(TCH):
                ps = psqk.tile([128, 512], F32, tag="qk")
                for dc in range(DCH):
                    nc.tensor.matmul(
                        ps,
                        wqk_sb[:, dc, ts(ct, 128)].bitcast(MM),
                        xt_sb[:, dc, ts(tc_, 512)].bitcast(MM),
                        start=(dc == 0),
                        stop=(dc == DCH - 1),
                    )
                nc.scalar.copy(qk_sb[:, ct, ts(tc_, 512)], ps)

        for tt in range(TT):
            pv = psv.tile([128, 64], F32, tag="v")
            for dc in range(DCH):
                nc.tensor.matmul(
                    pv,
                    xt_sb[:, dc, ts(tt, 128)].bitcast(MM),
                    wv_sb[:, dc, :].bitcast(MM),
                    start=(dc == 0),
                    stop=(dc == DCH - 1),
                )
            nc.vector.tensor_copy(v_sb[:, tt, 0:64], pv)
            nc.vector.memset(v_sb[:, tt, 64:65].bitcast(F32), 1.0)

    # ---- stage C: attention ----
    with (
        tc.tile_pool(name="psS", bufs=2, space="PSUM") as psS,
        tc.tile_pool(name="psPV", bufs=2, space="PSUM") as psPV,
        tc.tile_pool(name="et", bufs=6) as et_pool,
        tc.tile_pool(name="nrm", bufs=4) as nrm_pool,
    ):
        for m in range(NPAIR):
            for c in range(TCH):
                pv0 = psPV.tile([65, 512], F32, tag="pv0")
                pv1 = psPV.tile([65, 512], F32, tag="pv1")
                pvt = [pv0, pv1]
                n_i = 4 * c + 4
                for i in range(n_i):
                    r = i - 4 * c
                    diag = 0 <= r < 4
                    for h in (0, 1):
                        p0 = 64 * h
                        ss = psS.tile([128, 512], F32, tag=f"s{h}")
                        nc.tensor.matmul(
                            ss,
                            qk_sb[p0 : p0 + 64, 4 + m, ts(i, 128)].bitcast(MM),
                            qk_sb[p0 : p0 + 64, m, ts(c, 512)].bitcast(MM),
                            start=True,
                            stop=True,
                        )
                        et = et_pool.tile([128, 512], MM, tag=f"e{h}")
                        if diag:
                            lo = 128 * r
                            if lo > 0:
                                nc.vector.memset(et[:, 0:lo].bitcast(F32), 0.0)
                            nc.scalar.activation(
                                et[:, lo:512], ss[:, lo:512], Exp, scale=0.125
                            )
                            nc.vector.tensor_mul(
                                et[:, lo : lo + 128], et[:, lo : lo + 128], mask
                            )
                        else:
                            nc.scalar.activation(et, ss, Exp, scale=0.125)
                        nc.tensor.matmul(
                            pvt[h],
                            v_sb[:, i, 0:65].bitcast(MM),
                            et.bitcast(MM),
                            start=(i == 0),
                            stop=(i == n_i - 1),
                        )
                for h in (0, 1):
                    pv = pvt[h]
                    rcp = nrm_pool.tile([1, 512], MM, tag="rcp")
                    nc.vector.reciprocal(rcp, pv[64:65, :])
                    # broadcast rcp across 64 partitions: K=1 matmul with ones
                    rb_ps = psS.tile([64, 512], F32, tag=f"s{h}")
                    nc.tensor.matmul(
                        rb_ps,
                        ones64.bitcast(MM),
                        rcp.bitcast(MM),
                        start=True,
                        stop=True,
                    )
                    rb = nrm_pool.tile([64, 512], F32, tag="rb")
                    nc.scalar.copy(rb, rb_ps)
                    tmp = nrm_pool.tile([64, 512], F32, tag="tmp")
                    nc.vector.tensor_mul(tmp, pv[0:64, :], rb)
                    nc.vector.tensor_add(
                        acc[:, ts(c, 512)], acc[:, ts(c, 512)], tmp
                    )

    # ---- stage D: output projection ----
    with (
        tc.tile_pool(name="psD", bufs=4, space="PSUM") as psD,
        tc.tile_pool(name="ot", bufs=4) as ot_pool,
    ):
        for tt in range(TT):
            for mc in range(2):
                po = psD.tile([128, 512], F32, tag="o")
                nc.tensor.matmul(
                    po,
                    acc[:, ts(tt, 128)].bitcast(MM),
                    wout_sb[:, ts(mc, 512)].bitcast(MM),
                    start=True,
                    stop=True,
                )
                ot = ot_pool.tile([128, 512], F32, tag="ot")
                nc.vector.tensor_copy(ot, po)
                nc.sync.dma_start(
                    out=out[ts(tt, 128), ts(mc, 512)], in_=ot
                )


_NC_CACHE = [None]


def build_nc():
    if _NC_CACHE[0] is not None:
        return _NC_CACHE[0]
    nc = bass.Bass("TRN2", target_bir_lowering=False, debug=False)
    xT = nc.declare_dram_parameter("xT", [D, T], MM, isOutput=False)
    wqk = nc.declare_dram_parameter("wqk", [D, 1024], MM, isOutput=False)
    wv = nc.declare_dram_parameter("wv", [D, 64], MM, isOutput=False)
    wout = nc.declare_dram_parameter("wout", [64, D], MM, isOutput=False)
    out = nc.declare_dram_parameter("out", [T, D], F32, isOutput=True)
    with tile.TileContext(nc) as tc, nc.allow_low_precision(
        reason="f32r (TF32) matmul path; verified ~2e-4 rel err vs fp32 ref"
    ):
        _emit_body(nc, tc, xT, wqk, wv, wout, out)
    _split_multiwaits(nc, maxw=1)
    _NC_CACHE[0] = nc
    return nc


def make_in_maps(x, W_qkv, W_out):
    wv = np.ascontiguousarray(W_qkv[:, 2 * H * DH :], dtype=np.float32)
    wout = np.ascontiguousarray(W_out / float(H), dtype=np.float32)
    in_maps = []
    for core in range(N_CORES):
        b, hg = core // 2, core % 2
        xT = np.ascontiguousarray(x[b].T, dtype=np.float32)
        cols = []
        for off in (0, H * DH):  # q block then k block
            for mp in range(NPAIR):
                h0 = hg * HPC + 2 * mp
                cols.append(W_qkv[:, off + h0 * DH : off + (h0 + 2) * DH])
        wqk = np.ascontiguousarray(np.concatenate(cols, axis=1), dtype=np.float32)
        in_maps.append({"xT": xT, "wqk": wqk, "wv": wv, "wout": wout})
    return in_maps


def kernel(x, W_qkv, W_out, _trace=False, _trace_kwargs=None):
    nc = build_nc()
    in_maps = make_in_maps(x, W_qkv, W_out)
    res = run_bass_kernel_spmd(
        nc, in_maps, list(range(N_CORES)), trace=_trace, **(_trace_kwargs or {})
    )
    out = np.empty((B, T, D), dtype=np.float32)
    for b in range(B):
        out[b] = res.results[2 * b]["out"] + res.results[2 * b + 1]["out"]
    if _trace:
        return out, res
    return out



# revision 25
# speedup vs baseline: 1.8632x; 1.8632x over previous
"""InterpretableMultiHeadAttention on 8 Trainium2 NeuronCores.

Model (reference): qkv = x @ W_qkv; 16 q/k heads of 64, one shared v head;
causal softmax attention per head with shared V; mean over heads; @ W_out.

Sharding: core = (batch b, head-group hg of 8 heads). Each core computes its
8 heads' attention for its batch, applies (W_out/16) to the head-sum, and the
host adds the two head-group partials per batch.

v2 design (Act-engine-bound pipeline, all-bf16 data path, ~3e-3 rel err):
  - bf16 everywhere on the PE: every matmul is 1 cycle/row of output free dim
  - scores [s=128, t=512] into 2-bank psum pair-tiles [128, 2, 512] (2 heads);
    ONE activation (exp, scale=1/8) per pair covering only [lo:512] on diag
    tiles -> the Act engine runs exp and nothing else (it is the bottleneck)
  - causal diag masking post-exp on DVE (tensor_mul by 0/1 upper-tri, bf16)
  - PV flipped: psum [t=128, 65] <- lhsT = expT s-chunk slice, rhs = v_aug
    [s, 64+ones] -> denominator rides as column 64; free dim 65, not 512
  - normalization: per-partition reciprocal + one fused scalar_tensor_tensor
    (acc[t,dh] += attn * rcp) on DVE; no broadcast matmul, no Act copies
  - all PSUM->SBUF copies on gpsimd (Pool)
  - projections (qk, v) interleaved into the attention stream one pair ahead
    so the PE never gives the Act engine a gap; PE warmup matmuls cover the
    input-DMA window to keep the p-state ramp
  - out projection (PE-transposed acc tiles) emitted per c-group during the
    last pair's attention; bf16 output DMA; host upcasts + adds partials
"""

import numpy as np
import ml_dtypes

import concourse.bass as bass
import concourse.mybir as mybir
import concourse.tile as tile
from concourse.bass_utils import run_bass_kernel_spmd
from concourse.masks import (
    make_identity,
    make_lower_triangular,
    make_upper_triangular,
)

F32 = mybir.dt.float32
BF16 = mybir.dt.bfloat16
MM = mybir.dt.float32r

B, T, D = 4, 2048, 1024
H, DH = 16, 64
HPC = 8          # heads per core
NPAIR = HPC // 2
DCH = D // 128   # 8 contraction chunks
TCH = T // 512   # 4 free-dim chunks
TT = T // 128    # 16 token tiles
N_CORES = 8
N_WARM = 8       # PE warmup matmuls before the prologue

_uid = [0]


def _split_multiwaits(nc, maxw=1):
    """walrus rejects instructions with multiple sync waits (observed on the
    Tile exit drain). Move extra waits onto same-engine NoOps just before."""
    for _name, bbh in nc.bb_map.items():
        bb = bbh.bb if hasattr(bbh, "bb") else bbh
        insts = bb.instructions
        new = []
        for inst in insts:
            si = inst.sync_info
            if si is not None and len(si.on_wait) > maxw:
                waits = list(si.on_wait)
                extra, keep = waits[:-maxw], waits[-maxw:]
                for k in range(0, len(extra), maxw):
                    _uid[0] += 1
                    nop = mybir.InstNoOp(
                        name=f"I-waitsplit-{_uid[0]}", ins=[], outs=[]
                    )
                    nop.engine = inst.engine
                    nop.sync_info = mybir.SyncInfo(
                        on_wait=extra[k : k + maxw], on_update=[]
                    )
                    new.append(nop)
                inst.sync_info = mybir.SyncInfo(
                    on_wait=keep, on_update=list(si.on_update)
                )
            new.append(inst)
        insts[:] = new


def _emit_body(nc, tc, xT, wqk1, wqk2, wv, wout, out):
    Exp = mybir.ActivationFunctionType.Exp
    Mult = mybir.AluOpType.mult
    Add = mybir.AluOpType.add
    ts = bass.ts

    from contextlib import ExitStack

    _ctx = ExitStack()
    consts = _ctx.enter_context(tc.tile_pool(name="consts", bufs=1))
    ident_bf = consts.tile([128, 128], BF16)
    make_identity(nc, ident_bf)
    mask01 = consts.tile([128, 128], BF16)   # 1.0 where col >= row else 0
    make_upper_triangular(nc, mask01, val=1.0, diag=True)
    maskneg = consts.tile([128, 128], BF16)  # -30000 where col < row else 0
    make_lower_triangular(nc, maskneg, val=-30000.0, diag=False)
    ident_f = consts.tile([128, 128], F32)
    make_identity(nc, ident_f)
    wout_sb = consts.tile([64, D], F32)
    wsrc = consts.tile([128, 512], BF16)       # warmup matmul source
    nc.vector.memset(wsrc, 0.5)
    v_sb = consts.tile([128, TT, 65], BF16)    # v tiles + ones column
    nc.vector.memset(v_sb[:, :, 64:65], 1.0)
    acc = consts.tile([128, TT, 64], F32)      # sum_h attn_h/denom_h, [t, dh]
    nc.vector.memset(acc, 0.0)
    qk_sb = consts.tile([128, 8, T], BF16)     # groups: q pairs 0-3, k pairs 4-7
    accT_sb = consts.tile([64, TT, 128], F32)  # transposed acc for out proj

    # wqk block layout (halves as DMA'd): [q0 k0 q1 k1 | q2 q3 k2 k3]
    pair_blks = {0: (0, 1), 1: (2, 3), 2: (4, 6), 3: (5, 7)}

    with (
        tc.tile_pool(name="xt", bufs=1) as xt_pool,
        tc.tile_pool(name="w", bufs=1) as w_pool,
        tc.tile_pool(name="psS", bufs=2, space="PSUM") as psS,
        tc.tile_pool(name="psB", bufs=2, space="PSUM") as psB,
        tc.tile_pool(name="psPV", bufs=2, space="PSUM") as psPV,
        tc.tile_pool(name="et", bufs=36) as et_pool,
        tc.tile_pool(name="nrm", bufs=8) as nrm_pool,
        tc.tile_pool(name="osb", bufs=3) as osb_pool,
    ):
        wqk_sb = w_pool.tile([128, DCH, 1024], BF16)
        wv_sb = w_pool.tile([128, DCH, 64], BF16)
        xt_sb = xt_pool.tile([128, DCH, T], BF16)
        nc.sync.dma_start(out=wqk_sb[:, :, 0:256], in_=wqk1[:, :, 0:256])
        for dc in range(DCH):
            nc.sync.dma_start(out=xt_sb[:, dc, :], in_=xT[:, dc, :])
        nc.sync.dma_start(out=wqk_sb[:, :, 256:512], in_=wqk1[:, :, 256:512])
        nc.sync.dma_start(out=wv_sb, in_=wv[:])
        nc.sync.dma_start(out=wqk_sb[:, :, 512:1024], in_=wqk2[:])
        nc.sync.dma_start(out=wout_sb, in_=wout[:])

        def warm(n):
            # PE busy-work for DMA-wait windows: keeps the p-state ramp hot.
            for _ in range(n):
                wm = psPV.tile([128, 512], F32, tag="pv", name="wm")
                nc.tensor.matmul(wm, ident_bf, wsrc, start=True, stop=True)

        warm(N_WARM)

        def b_quarter(state, blk, grp, tc_, q, act=False):
            """Quarter of one qk-projection tile (2 dc chunks); q==3 copies.
            GPSIMD has no PSUM port: copies go to DVE (or Act in the
            prologue, where the Act engine is still idle)."""
            if q == 0:
                state["ps"] = psB.tile([128, 512], F32, tag="b", name="psb")
            ps = state["ps"]
            for dc in (2 * q, 2 * q + 1):
                nc.tensor.matmul(
                    ps,
                    wqk_sb[:, dc, ts(blk, 128)],
                    xt_sb[:, dc, ts(tc_, 512)],
                    start=(dc == 0),
                    stop=(dc == DCH - 1),
                )
            if q == 3:
                if act:
                    nc.scalar.copy(qk_sb[:, grp, ts(tc_, 512)], ps)
                else:
                    nc.vector.tensor_copy(qk_sb[:, grp, ts(tc_, 512)], ps)

        def b_unit(blk, grp, tc_):
            state = {}
            for q in range(4):
                b_quarter(state, blk, grp, tc_, q)

        def v_half(state, tt, hf):
            if hf == 0:
                state["pv"] = psPV.tile([128, 65], F32, tag="pv", name="psvv")
            pv = state["pv"][:, 0:64]
            for dc in range(4 * hf, 4 * hf + 4):
                nc.tensor.matmul(
                    pv,
                    xt_sb[:, dc, ts(tt, 128)],
                    wv_sb[:, dc, :],
                    start=(dc == 0),
                    stop=(dc == DCH - 1),
                )
            if hf == 1:
                nc.vector.tensor_copy(v_sb[:, tt, 0:64], state["pv"][:, 0:64])

        ets = {}

        def s_tile(m, c, i, pe_mask=False):
            """Scores + exp for s-chunk i of group (m, c), both heads.
            Diagonal masking: -30000 added in PSUM via matmul (pe_mask, for
            groups where the PE has slack) or 0/1 multiply post-exp on DVE."""
            r = i - 4 * c
            lo = 128 * r if r >= 0 else 0
            ps = psS.tile([128, 2, 512], F32, tag="s", name="pss")
            for hh in (0, 1):
                p0 = 64 * hh
                nc.tensor.matmul(
                    ps[:, hh, lo:512],
                    qk_sb[p0 : p0 + 64, 4 + m, ts(i, 128)],
                    qk_sb[p0 : p0 + 64, m, 512 * c + lo : 512 * c + 512],
                    start=True,
                    stop=(r < 0 or not pe_mask),
                    skip_group_check=True,
                )
                if r >= 0 and pe_mask:
                    nc.tensor.matmul(
                        ps[:, hh, lo : lo + 128],
                        ident_bf,
                        maskneg,
                        start=False,
                        stop=True,
                        skip_group_check=True,
                    )
            et = et_pool.tile([128, 2, 512], BF16, tag="et", name="et")
            nc.scalar.activation(
                et[:, :, lo:512], ps[:, :, lo:512], Exp, scale=0.125
            )
            if r >= 0 and not pe_mask:
                for hh in (0, 1):
                    nc.vector.tensor_mul(
                        et[:, hh, lo : lo + 128], et[:, hh, lo : lo + 128], mask01
                    )
            ets.setdefault((m, c), []).append(et)

        tailpool = [0]

        def tail_ps(n, wide=False):
            """Rotate tail psum allocs across idle pools: psB+psPV during m3
            (psB has no b-units left), plus psS in the final flush."""
            pools = (psS, psB, psPV) if wide else (psB, psPV)
            tailpool[0] = (tailpool[0] + 1) % len(pools)
            p = pools[tailpool[0]]
            if p is psS:
                t = psS.tile([128, 2, 512], F32, tag="s", name="tps")
                return t[:, 0, 0:n]
            t = p.tile([128, 512], F32, tag="b" if p is psB else "pv", name="tps")
            return t[:, 0:n]

        def pv_seq(m, c, jj, hh, tail=False, wide=False):
            """Attention output for t-tile 4c+jj, head (2m+hh): PV + norm."""
            j = 4 * c + jj
            lst = ets[(m, c)]
            if tail:
                pv = tail_ps(65, wide)
            else:
                pv = psPV.tile([128, 65], F32, tag="pv", name="pspv")
            for i in range(j + 1):
                nc.tensor.matmul(
                    pv,
                    lst[i][:, hh, ts(jj, 128)],
                    v_sb[:, i, 0:65],
                    start=(i == 0),
                    stop=(i == j),
                )
            rcp = nrm_pool.tile([128, 1], F32, tag="rcp", name="rcp")
            nc.vector.reciprocal(rcp, pv[:, 64:65])
            nc.vector.scalar_tensor_tensor(
                acc[:, j, :], pv[:, 0:64], rcp, acc[:, j, :], Mult, Add
            )

        def out_unit(j, tail=False, act_copy=False, wide=False):
            """Out projection + DMA for t-tile j (all heads accumulated)."""
            pst = tail_ps(128, wide) if tail else psPV.tile(
                [128, 128], F32, tag="pv", name="psst"
            )
            nc.tensor.transpose(pst[0:64, :], acc[:, j, :], ident_f)
            nc.vector.tensor_copy(accT_sb[:, j, :], pst[0:64, :])
            for mc in range(2):
                po = tail_ps(512, wide) if tail else psPV.tile(
                    [128, 512], F32, tag="pv", name="psso"
                )
                nc.tensor.matmul(
                    po,
                    accT_sb[:, j, :].bitcast(MM),
                    wout_sb[:, ts(mc, 512)].bitcast(MM),
                    start=True,
                    stop=True,
                )
                osb = osb_pool.tile([128, 512], BF16, tag="o", name="osb")
                if act_copy and mc == 0:
                    nc.scalar.copy(osb, po)
                else:
                    nc.vector.tensor_copy(osb, po)
                nc.sync.dma_start(
                    out=out[ts(j, 128), ts(mc, 512)], in_=osb
                )

        # ---- group order: m-major, but m3 runs [c1 c2 c3 c0] so the final
        # group is small (4 exp tiles) -> short pipeline drain.
        order = [(m, c) for m in range(3) for c in range(TCH)] + [
            (3, 1), (3, 2), (3, 3), (3, 0)
        ]
        # projection-unit placement: (group) -> list of (pair, isk, tc_) with
        # every unit emitted before the first S tile that reads it.
        UNITS = {
            (0, 0): [(0, 0, 1), (0, 1, 1)],
            (0, 1): [(0, 0, 2), (0, 1, 2)],
            (0, 2): [(0, 0, 3), (0, 1, 3)],
            (0, 3): [(1, 0, 0), (1, 1, 0)],
            (1, 0): [(1, 0, 1), (1, 1, 1)],
            (1, 1): [(1, 0, 2), (1, 1, 2)],
            (1, 2): [(1, 0, 3), (1, 1, 3)],
            (1, 3): [(2, 0, 0), (2, 1, 0)],
            (2, 0): [(2, 0, 1), (2, 1, 1)],
            (2, 1): [(2, 0, 2), (2, 1, 2), (3, 1, 0)],
            (2, 2): [(2, 0, 3), (2, 1, 3), (3, 0, 1), (3, 1, 1)],
            (2, 3): [(3, 0, 2), (3, 1, 2), (3, 0, 3), (3, 1, 3), (3, 0, 0)],
        }

        # prologue: only pair-0 tc0 (q then k), needed by S(0,0); warmups
        # fill the xt-chunk DMA wait between quarters
        q0, k0 = pair_blks[0]
        stq, stk = {}, {}
        for q in range(4):
            b_quarter(stq, q0, 0, 0, q, act=True)
            b_quarter(stk, k0, 4, 0, q, act=True)
            if q < 3:
                warm(6)

        for g, (m, c) in enumerate(order):
            filler = []
            for pair, isk, tcu in UNITS.get((m, c), ()):
                qb, kb = pair_blks[pair]
                bb = kb if isk else qb
                gg = (4 + pair) if isk else pair
                st = {}
                for q in range(4):
                    filler.append((b_quarter, (st, bb, gg, tcu, q, isk == 0)))
            if m == 0:  # v tiles, 4 per c
                for tt in range(4 * c, 4 * c + 4):
                    st = {}
                    for hf in range(2):
                        filler.append((v_half, (st, tt, hf)))
            if g > 0:
                pm, pc = order[g - 1]
                m3chain = pm == NPAIR - 1
                for jj in range(4):
                    filler.append((pv_seq, (pm, pc, jj, 0, m3chain)))
                    filler.append((pv_seq, (pm, pc, jj, 1, m3chain)))
                    if m3chain:
                        filler.append((out_unit, (4 * pc + jj, True)))

            last = g == len(order) - 1
            n_s = 4 * c + 4
            taken = 0
            for i in range(n_s):
                s_tile(m, c, i, pe_mask=(m == NPAIR - 1))
                if last:
                    continue  # final group: all S first so Act finishes ASAP
                want = (len(filler) * (i + 1)) // n_s
                while taken < want:
                    fn, args = filler[taken]
                    fn(*args)
                    taken += 1
            if last:
                while taken < len(filler):
                    fn, args = filler[taken]
                    fn(*args)
                    taken += 1
                for jj in range(4):
                    pv_seq(m, c, jj, 0, True, True)
                    pv_seq(m, c, jj, 1, True, True)
                    out_unit(4 * c + jj, True, True, True)
            while taken < len(filler):
                fn, args = filler[taken]
                fn(*args)
                taken += 1
            if g > 0:
                ets.pop(order[g - 1], None)
        ets.pop(order[-1], None)


_NC_CACHE = [None]


def build_nc():
    if _NC_CACHE[0] is not None:
        return _NC_CACHE[0]
    nc = bass.Bass("TRN2", target_bir_lowering=False, debug=False)
    xT = nc.declare_dram_parameter("xT", [128, DCH, T], BF16, isOutput=False)
    wqk1 = nc.declare_dram_parameter("wqk1", [128, DCH, 512], BF16, isOutput=False)
    wqk2 = nc.declare_dram_parameter("wqk2", [128, DCH, 512], BF16, isOutput=False)
    wv = nc.declare_dram_parameter("wv", [128, DCH, 64], BF16, isOutput=False)
    wout = nc.declare_dram_parameter("wout", [64, D], F32, isOutput=False)
    out = nc.declare_dram_parameter("out", [T, D], BF16, isOutput=True)
    with tile.TileContext(nc) as tc, nc.allow_low_precision(
        reason="bf16 data path; measured ~3e-3 rel err vs fp32 ref (tol 2e-2)"
    ):
        _emit_body(nc, tc, xT, wqk1, wqk2, wv, wout, out)
    _split_multiwaits(nc, maxw=1)
    _NC_CACHE[0] = nc
    return nc


def make_in_maps(x, W_qkv, W_out):
    bf = ml_dtypes.bfloat16

    def chunked(a):
        cols = a.shape[1]
        return np.ascontiguousarray(
            a.reshape(DCH, 128, cols).transpose(1, 0, 2).astype(bf)
        )

    wv = chunked(np.asarray(W_qkv[:, 2 * H * DH :], dtype=np.float32))
    wout = np.ascontiguousarray(np.asarray(W_out, dtype=np.float32) / float(H))
    in_maps = []
    for core in range(N_CORES):
        b, hg = core // 2, core % 2
        xT = chunked(np.ascontiguousarray(x[b].T, dtype=np.float32))
        halves = []
        for half in range(2):
            cols = []
            if half == 0:  # interleaved [q0, k0, q1, k1] so pair0 loads first
                for mp in (0, 1):
                    h0 = hg * HPC + 2 * mp
                    for off in (0, H * DH):
                        cols.append(W_qkv[:, off + h0 * DH : off + (h0 + 2) * DH])
            else:  # [q2, q3, k2, k3]
                for off in (0, H * DH):
                    for mp in (2, 3):
                        h0 = hg * HPC + 2 * mp
                        cols.append(W_qkv[:, off + h0 * DH : off + (h0 + 2) * DH])
            halves.append(chunked(np.concatenate(cols, axis=1)))
        in_maps.append(
            {
                "xT": xT,
                "wqk1": halves[0],
                "wqk2": halves[1],
                "wv": wv,
                "wout": wout,
            }
        )
    return in_maps


def kernel(x, W_qkv, W_out, _trace=False, _trace_kwargs=None):
    nc = build_nc()
    in_maps = make_in_maps(x, W_qkv, W_out)
    res = run_bass_kernel_spmd(
        nc, in_maps, list(range(N_CORES)), trace=_trace, **(_trace_kwargs or {})
    )
    out = np.empty((B, T, D), dtype=np.float32)
    for b in range(B):
        out[b] = res.results[2 * b]["out"].astype(np.float32) + res.results[
            2 * b + 1
        ]["out"].astype(np.float32)
    if _trace:
        return out, res
    return out
